# revision 14
# baseline (speedup 1.0000x reference)
"""Trainium2 Bass kernel for voxel-CNN + point-MLP (nn_CNN_Baseline_62646392980178).

Sharding: data-parallel over batch B=8 across 8 NeuronCores (one sample per
core); params replicated.

Front-end (this rewrite): instead of packing 256B neighborhood rows on the
host (512MB of numpy + ~1GB host->device traffic), each core receives only
the raw zero-padded voxel grid (bf16 [4,66,66,66], 2.3MB).  On device, the
full-grid conv runs as im2col built by 27 structured DMAs per voxel chunk
(one per 3x3x3 tap, 4 input channels each) followed by one [108->32] matmul.
Conv columns are PE-transposed to per-voxel 256B rows and dma_scatter_add'ed
into a small DRAM table at "first point of this voxel" slots (idx -1 = no
point -> skipped).  A single transpose-mode dma_gather (idx = first-point
slot of each sorted point, which also resolves duplicate-voxel points) then
yields conv features as [channel, point] columns feeding the MLP directly.

Back-end (unchanged from baseline): MLP (128/128/256/10) on TensorE over the
sorted points; training-mode BatchNorm over (B, P) via per-channel sum
allreduce across the 8 cores; the global-max feature's layer-1 contribution
is folded into the BN statistics and layer-1 bias (cross terms in the
allreduce payload), removing the max-pool barrier from the matmul pipeline.
"""

import os
import sys

sys.path.insert(0, "/opt/trn_rl_repo")

import numpy as np
import ml_dtypes

import concourse.bass as bass
import concourse.bacc as bacc
import concourse.mybir as mybir
import concourse.tile as tile
from concourse.bass_utils import run_bass_kernel_spmd

BF16 = ml_dtypes.bfloat16
F32 = mybir.dt.float32
BF = mybir.dt.bfloat16
I16 = mybir.dt.int16
AF = mybir.ActivationFunctionType
OP = mybir.AluOpType

GRID = 64
PGRID = GRID + 2
NVOX = GRID ** 3
EPS = 1e-5
B = 8
P = 16384
CIN = 4
COUT = 32
K_IM = 108
ROW = 128             # bf16 elems per table row (256B)
NSLOT = 32768         # conv table rows; [P, 32768) = trash (never gathered)
CHUNK = 2048          # voxels per conv chunk (half a z-plane)
NCHK = NVOX // CHUNK  # 128
YROWS = CHUNK // GRID  # 32 y-rows per chunk
NCH = P // 512        # 32 MLP chunks
D1, D2, D3, DO = 128, 128, 256, 10
N_TOT = float(B * P)

_prog_cache = [None]
_runner_cache = [None]
_REPLICATED = frozenset([
    "ident", "wext", "b3d", "w1pt", "w1cv", "w1gl", "w2", "w3a", "w3b",
    "wo_a", "wo_b", "gb1", "gb2", "gb3a", "gb3b", "bout"])


def _run_cached(nc, in_maps):
    """Dispatch the prebuilt Bass module via a PERSISTENT jitted callable.

    run_bass_kernel_spmd -> run_bass_via_pjrt builds a fresh jax.jit wrapper
    on every call (full retrace + XLA pipeline each time, ~1s).  This caches
    the sharded executable across kernel() calls; logic mirrors
    bass2jax.run_bass_via_pjrt's multi-core branch.
    """
    import jax
    from jax.experimental.shard_map import shard_map
    from jax.sharding import Mesh, PartitionSpec
    from concourse import bass2jax as B2J

    if _runner_cache[0] is None:
        B2J.install_neuronx_cc_hook()
        assert nc.dbg_addr is None, "cached runner assumes debug=False"
        partition_name = (nc.partition_id_tensor.name
                          if nc.partition_id_tensor else None)
        in_names, out_names, out_avals = [], [], []
        for alloc in nc.m.functions[0].allocations:
            if not isinstance(alloc, mybir.MemoryLocationSet):
                continue
            name = alloc.memorylocations[0].name
            if alloc.kind == "ExternalInput":
                if name != partition_name:
                    in_names.append(name)
            elif alloc.kind == "ExternalOutput":
                out_names.append(name)
                out_avals.append(jax.core.ShapedArray(
                    tuple(alloc.tensor_shape), mybir.dt.np(alloc.dtype)))
        n_params, n_outs = len(in_names), len(out_avals)
        all_names = list(in_names) + list(out_names)
        if partition_name is not None:
            all_names.append(partition_name)
        donate = tuple(range(n_params, n_params + n_outs))

        def _body(*args):
            operands = list(args)
            if partition_name is not None:
                operands.append(B2J.partition_id_tensor())
            return tuple(B2J._bass_exec_p.bind(
                *operands, out_avals=tuple(out_avals),
                in_names=tuple(all_names), out_names=tuple(out_names),
                lowering_input_output_aliases=(),
                sim_require_finite=True, sim_require_nnan=True, nc=nc))

        devices = jax.devices()[:B]
        mesh = Mesh(np.asarray(devices), ("core",))
        in_specs = tuple(
            PartitionSpec() if name in _REPLICATED else PartitionSpec("core")
            for name in in_names) + (PartitionSpec("core"),) * n_outs
        sharded = jax.jit(
            shard_map(_body, mesh=mesh, in_specs=in_specs,
                      out_specs=(PartitionSpec("core"),) * n_outs,
                      check_rep=False),
            donate_argnums=donate, keep_unused=True)
        _runner_cache[0] = (sharded, in_names, out_names, out_avals)

    sharded, in_names, out_names, out_avals = _runner_cache[0]
    concat_in = [
        in_maps[0][name] if name in _REPLICATED
        else np.concatenate([np.asarray(m[name]) for m in in_maps], axis=0)
        for name in in_names]
    concat_zeros = [np.zeros((B * a.shape[0], *a.shape[1:]), a.dtype)
                    for a in out_avals]
    out_arrs = sharded(*concat_in, *concat_zeros)
    return [{name: np.asarray(out_arrs[i]).reshape(B, *out_avals[i].shape)[c]
             for i, name in enumerate(out_names)}
            for c in range(B)]


def _build_program():
    nc = bacc.Bacc("TRN2", target_bir_lowering=False, debug=False, num_devices=8)

    def din(name, shape, dt):
        return nc.dram_tensor(name, shape, dt, kind="ExternalInput").ap()

    xpad = din("xpad", [CIN, PGRID, PGRID, PGRID], BF)
    sidx = din("sidx", [16, NCHK, CHUNK // 16], I16)
    gidx = din("gidx", [128, P // 16], I16)
    ptT = din("ptT", [6, P], BF)
    ident = din("ident", [128, 128], BF)
    wext = din("wext", [K_IM, COUT], BF)
    b3dp = din("b3d", [COUT, 1], F32)
    w1pt = din("w1pt", [6, D1], BF)
    w1cv = din("w1cv", [COUT, D1], BF)
    w1gl = din("w1gl", [COUT, D1], BF)
    w2p = din("w2", [D1, D2], BF)
    w3ap = din("w3a", [D2, 128], BF)
    w3bp = din("w3b", [D2, 128], BF)
    woap = din("wo_a", [128, DO], BF)
    wobp = din("wo_b", [128, DO], BF)
    gb1p = din("gb1", [D1, 2], F32)
    gb2p = din("gb2", [D2, 2], F32)
    gb3ap = din("gb3a", [128, 2], F32)
    gb3bp = din("gb3b", [128, 2], F32)
    boutp = din("bout", [128, 8, DO], F32)
    out = nc.dram_tensor("out", [128, P // 128, DO], F32, kind="ExternalOutput").ap()

    _stage = os.environ.get("K_STAGE", "full")
    _skip_cc = os.environ.get("K_SKIP_CC", "0") == "1"

    with tile.TileContext(nc) as tc:
        with tc.tile_pool(name="sb", bufs=1) as sb, \
             tc.tile_pool(name="ps", bufs=2, space="PSUM") as ps, \
             tc.tile_pool(name="psy", bufs=1, space="PSUM") as psy, \
             tc.tile_pool(name="imp", bufs=2) as imp, \
             tc.tile_pool(name="ssp", bufs=2) as ssp, \
             tc.tile_pool(name="sxp", bufs=2) as sxp, \
             tc.tile_pool(name="dramp", bufs=1, space="DRAM") as dramp:

            table = dramp.tile([NSLOT, ROW], BF, tag="table")

            gidx_sb = sb.tile([128, P // 16], I16, tag="gidx")
            nc.sync.dma_start(out=gidx_sb[:], in_=gidx[:])

            id_sb = sb.tile([128, 128], BF, tag="ident")
            nc.sync.dma_start(out=id_sb[:], in_=ident[:])

            def loadw(ap_, shape, dt, tag):
                t = sb.tile(shape, dt, tag=tag)
                nc.sync.dma_start(out=t[:], in_=ap_[:])
                return t

            wext_sb = loadw(wext, [K_IM, COUT], BF, "wext")
            b3d_sb = loadw(b3dp, [COUT, 1], F32, "b3d")
            w1pt_sb = loadw(w1pt, [6, D1], BF, "w1pt")
            w1cv_sb = loadw(w1cv, [COUT, D1], BF, "w1cv")
            w1gl_sb = loadw(w1gl, [COUT, D1], BF, "w1gl")
            w2_sb = loadw(w2p, [D1, D2], BF, "w2")
            w3a_sb = loadw(w3ap, [D2, 128], BF, "w3a")
            w3b_sb = loadw(w3bp, [D2, 128], BF, "w3b")
            woa_sb = loadw(woap, [128, DO], BF, "woa")
            wob_sb = loadw(wobp, [128, DO], BF, "wob")
            gb1_sb = loadw(gb1p, [D1, 2], F32, "gb1")
            gb2_sb = loadw(gb2p, [D2, 2], F32, "gb2")
            gb3a_sb = loadw(gb3ap, [128, 2], F32, "gb3a")
            gb3b_sb = loadw(gb3bp, [128, 2], F32, "gb3b")
            bout_sb = loadw(boutp, [128, 8, DO], F32, "bout")

            # ---- zero the scatter table (scatter_add accumulates) ----------
            zt = sb.tile([128, 1032], BF, tag="zt")
            nc.vector.memset(zt[:], 0.0)
            for i in range(16):
                nc.sync.dma_start(out=table[i * 1032:(i + 1) * 1032, :],
                                  in_=zt[:])

            # ---- full-grid conv -> per-voxel rows -> scatter to slots ------
            _nchk = int(os.environ.get("K_NCHK", str(NCHK)))
            for ck in range(_nchk if _stage != "mlponly" else 0):
                z = ck // 2
                y0 = (ck % 2) * YROWS
                im = imp.tile([K_IM, CHUNK], BF, tag="im")
                for dz in range(3):
                    for dy in range(3):
                        for dx in range(3):
                            k0 = ((dz * 3 + dy) * 3 + dx) * CIN
                            nc.sync.dma_start(
                                out=im[k0:k0 + CIN, :],
                                in_=xpad[0:CIN, z + dz,
                                         y0 + dy:y0 + dy + YROWS,
                                         dx:dx + GRID])
                ssrc = ssp.tile([128, CHUNK // 128, ROW], BF, tag="ssrc")
                if ck < 2:
                    nc.vector.memset(ssrc[:], 0.0)
                for i in range(CHUNK // 512):
                    cp = ps.tile([COUT, 512], F32, tag="cp")
                    nc.tensor.matmul(out=cp[:], lhsT=wext_sb[:],
                                     rhs=im[:, i * 512:(i + 1) * 512],
                                     start=True, stop=True)
                    cv = ssp.tile([COUT, 512], BF, tag="cv")
                    if i % 2 == 0:
                        nc.scalar.activation(out=cv[:], in_=cp[:], func=AF.Copy)
                    else:
                        nc.vector.tensor_copy(out=cv[:], in_=cp[:])
                    tp = ps.tile([128, 4, COUT], BF, tag="tp")
                    for t in range(4):
                        nc.tensor.transpose(
                            out=tp[:, t, :],
                            in_=cv[:, t * 128:(t + 1) * 128],
                            identity=id_sb[0:COUT, 0:COUT])
                    if i % 2 == 0:
                        nc.vector.tensor_copy(
                            out=ssrc[:, i * 4:(i + 1) * 4, 0:COUT], in_=tp[:])
                    else:
                        nc.scalar.activation(
                            out=ssrc[:, i * 4:(i + 1) * 4, 0:COUT], in_=tp[:],
                            func=AF.Copy)
                sx = sxp.tile([128, CHUNK // 16], I16, tag="sx")
                for j in range(8):
                    nc.sync.dma_start(out=sx[j * 16:(j + 1) * 16, :],
                                      in_=sidx[:, ck, :])
                for h in range(CHUNK // 512):
                    nc.gpsimd.dma_scatter_add(
                        out_ap=table[:], in_ap=ssrc[:, h * 4:(h + 1) * 4, :],
                        idxs_ap=sx[:, h * 32:(h + 1) * 32],
                        num_idxs=512, num_idxs_reg=512, elem_size=ROW,
                        queue_num=0)

            # ---- gather conv at sorted points: cpt[e, p] = table[gidx[p], e]
            cpt = sb.tile([128, 1, P], BF, tag="slot_cpt")
            if os.environ.get("K_NOGATHER", "0") != "1":
                for g in range(int(os.environ.get("K_NGCALLS", str(P // 512)))):
                    nc.gpsimd.dma_gather(
                        out_ap=cpt[:, :, g * 512:(g + 1) * 512],
                        in_ap=table[:],
                        idxs_ap=gidx_sb[:, g * 32:(g + 1) * 32],
                        num_idxs=512, num_idxs_reg=512, elem_size=ROW,
                        transpose=True, queue_num=0)
            else:
                nc.vector.memset(cpt[:], 0.25)
            conv_sb = cpt[0:COUT, 0, :]

            yraw = sb.tile([128, NCH, 512], F32, tag="slot_yraw")
            h1 = sb.tile([D1, P], BF, tag="slot_h13a")
            out_sb = sb.tile([128, P // 128, DO], F32, tag="outsb")

            if _stage == "conv":
                nc.vector.memset(out_sb[:], 0.0)
                nc.vector.tensor_copy(out=out_sb[:, 0:8, :].bitcast(BF),
                                      in_=cpt[:, 0, 0:160])
                nc.sync.dma_start(out=out[:], in_=out_sb[:])
            if _stage == "full":
                # ---- glob max -> v_g ---------------------------------------
                glob = sb.tile([COUT, 1], BF, tag="glob")
                nc.vector.tensor_reduce(out=glob[:], in_=conv_sb,
                                        axis=mybir.AxisListType.X, op=OP.max)
                nc.vector.tensor_scalar(out=glob[:], in0=glob[:],
                                        scalar1=b3d_sb[:, 0:1], scalar2=None,
                                        op0=OP.add)
                vgp = psy.tile([D1, 1], F32, tag="vg")
                nc.tensor.matmul(out=vgp[:], lhsT=w1gl_sb[:],
                                 rhs=glob[:], start=True, stop=True)
                vg = sb.tile([D1, 1], F32, tag="vgs")
                nc.vector.tensor_copy(out=vg[:], in_=vgp[:])

                # ---- L1 ----------------------------------------------------
                ptb = None
                for ch in range(NCH):
                    if ch % 4 == 0:
                        ptb = imp.tile([6, 2048], BF, tag="ptb")
                        nc.sync.dma_start(out=ptb[:],
                                          in_=ptT[:, ch * 512:(ch + 4) * 512])
                    yp = ps.tile([D1, 512], F32, tag="yp")
                    nc.tensor.matmul(out=yp[:],
                                     lhsT=w1pt_sb[:],
                                     rhs=ptb[:, (ch % 4) * 512:(ch % 4 + 1) * 512],
                                     start=True, stop=False)
                    nc.tensor.matmul(out=yp[:], lhsT=w1cv_sb[:],
                                     rhs=conv_sb[:, ch * 512:(ch + 1) * 512],
                                     start=False, stop=True)
                    if ch % 2 == 0:
                        nc.vector.tensor_copy(out=yraw[:, ch, :], in_=yp[:])
                    else:
                        nc.scalar.activation(out=yraw[:, ch, :], in_=yp[:],
                                             func=AF.Copy)

                # ---- helpers ----------------------------------------------
                def bn_sums(y_t, C, tag):
                    st = sb.tile([C, NCH * 6], F32, tag=tag + "st")
                    for ch in range(NCH):
                        nc.vector.bn_stats(out=st[:, ch * 6:(ch + 1) * 6],
                                           in_=y_t[:, ch, :])
                    ag = sb.tile([C, 2], F32, tag=tag + "ag")
                    nc.vector.bn_aggr(out=ag[:], in_=st[:])
                    s = sb.tile([C, 2], F32, tag=tag + "s")
                    nc.vector.tensor_tensor(out=s[:, 1:2], in0=ag[:, 0:1],
                                            in1=ag[:, 0:1], op=OP.mult)
                    nc.vector.tensor_tensor(out=s[:, 1:2], in0=s[:, 1:2],
                                            in1=ag[:, 1:2], op=OP.add)
                    nc.vector.tensor_scalar_mul(out=s[:, 1:2], in0=s[:, 1:2],
                                                scalar1=float(P))
                    nc.vector.tensor_scalar_mul(out=s[:, 0:1], in0=ag[:, 0:1],
                                                scalar1=float(P))
                    return s

                def allreduce(t_sb, shape, tag):
                    if _skip_cc:
                        r = sb.tile(shape, F32, tag=tag + "r")
                        nc.vector.tensor_scalar_mul(out=r[:], in0=t_sb[:],
                                                    scalar1=8.0)
                        return r
                    di = dramp.tile(shape, F32, tag=tag + "i")
                    do = dramp.tile(shape, F32, tag=tag + "o")
                    nc.gpsimd.dma_start(out=di[:], in_=t_sb[:])
                    nc.gpsimd.collective_compute(
                        "AllReduce", OP.add, replica_groups=[list(range(8))],
                        ins=[di.opt()], outs=[do.opt()])
                    r = sb.tile(shape, F32, tag=tag + "r")
                    nc.sync.dma_start(out=r[:], in_=do[:])
                    return r

                def bn_ab(red, gb, C, tag, vg_local=None):
                    a = sb.tile([C, 1], F32, tag=tag + "a")
                    cb = sb.tile([C, 1], F32, tag=tag + "c")
                    m = sb.tile([C, 1], F32, tag=tag + "m")
                    if vg_local is None:
                        nc.vector.tensor_scalar_mul(out=m[:], in0=red[:, 0:1],
                                                    scalar1=1.0 / N_TOT)
                        nc.vector.tensor_scalar_mul(out=a[:], in0=red[:, 1:2],
                                                    scalar1=1.0 / N_TOT)
                    else:
                        nc.vector.tensor_scalar_mul(out=m[:], in0=red[:, 4:5],
                                                    scalar1=float(P))
                        nc.vector.tensor_tensor(out=m[:], in0=m[:],
                                                in1=red[:, 0:1], op=OP.add)
                        nc.vector.tensor_scalar_mul(out=m[:], in0=m[:],
                                                    scalar1=1.0 / N_TOT)
                        t4 = sb.tile([C, 1], F32, tag=tag + "t4")
                        nc.vector.tensor_scalar_mul(out=a[:], in0=red[:, 2:3],
                                                    scalar1=2.0)
                        nc.vector.tensor_tensor(out=a[:], in0=a[:],
                                                in1=red[:, 1:2], op=OP.add)
                        nc.vector.tensor_scalar_mul(out=t4[:], in0=red[:, 3:4],
                                                    scalar1=float(P))
                        nc.vector.tensor_tensor(out=a[:], in0=a[:], in1=t4[:],
                                                op=OP.add)
                        nc.vector.tensor_scalar_mul(out=a[:], in0=a[:],
                                                    scalar1=1.0 / N_TOT)
                    msq = sb.tile([C, 1], F32, tag=tag + "q")
                    nc.vector.tensor_tensor(out=msq[:], in0=m[:], in1=m[:],
                                            op=OP.mult)
                    nc.vector.tensor_tensor(out=a[:], in0=a[:], in1=msq[:],
                                            op=OP.subtract)
                    nc.vector.tensor_scalar_add(out=a[:], in0=a[:], scalar1=EPS)
                    nc.scalar.activation(out=a[:], in_=a[:], func=AF.Sqrt)
                    nc.vector.reciprocal(out=a[:], in_=a[:])
                    nc.vector.tensor_tensor(out=a[:], in0=a[:], in1=gb[:, 0:1],
                                            op=OP.mult)
                    if vg_local is not None:
                        nc.vector.tensor_tensor(out=cb[:], in0=vg_local[:],
                                                in1=m[:], op=OP.subtract)
                        nc.vector.tensor_tensor(out=cb[:], in0=cb[:], in1=a[:],
                                                op=OP.mult)
                    else:
                        nc.vector.tensor_tensor(out=cb[:], in0=m[:], in1=a[:],
                                                op=OP.mult)
                        nc.vector.tensor_scalar_mul(out=cb[:], in0=cb[:],
                                                    scalar1=-1.0)
                    nc.vector.tensor_tensor(out=cb[:], in0=cb[:], in1=gb[:, 1:2],
                                            op=OP.add)
                    return a, cb

                def apply_relu(y_t, h_ap, a, cb):
                    hf = NCH // 2
                    nc.scalar.activation(out=h_ap[:, 0:P // 2],
                                         in_=y_t[:, 0:hf, :],
                                         func=AF.Relu, bias=cb[:], scale=a[:])
                    nc.vector.tensor_scalar(out=h_ap[:, P // 2:P],
                                            in0=y_t[:, hf:NCH, :],
                                            scalar1=a[:], scalar2=cb[:],
                                            op0=OP.mult, op1=OP.add)
                    nc.vector.tensor_scalar_max(out=h_ap[:, P // 2:P],
                                                in0=h_ap[:, P // 2:P],
                                                scalar1=0.0)

                # ---- BN1 ---------------------------------------------------
                s1 = bn_sums(yraw, D1, "b1")
                pk1 = sb.tile([D1, 5], F32, tag="pk1")
                nc.vector.tensor_copy(out=pk1[:, 0:2], in_=s1[:])
                nc.vector.tensor_tensor(out=pk1[:, 2:3], in0=vg[:],
                                        in1=s1[:, 0:1], op=OP.mult)
                nc.vector.tensor_tensor(out=pk1[:, 3:4], in0=vg[:], in1=vg[:],
                                        op=OP.mult)
                nc.vector.tensor_copy(out=pk1[:, 4:5], in_=vg[:])
                red1 = allreduce(pk1, [D1, 5], "r1")
                a1, c1 = bn_ab(red1, gb1_sb, D1, "x1", vg_local=vg)
                apply_relu(yraw, h1[:], a1, c1)

                # ---- L2 ----------------------------------------------------
                h2 = sb.tile([D2, P], BF, tag="slot_cpt")
                for ch in range(NCH):
                    yp = ps.tile([D2, 512], F32, tag="yp")
                    nc.tensor.matmul(out=yp[:], lhsT=w2_sb[:],
                                     rhs=h1[:, ch * 512:(ch + 1) * 512],
                                     start=True, stop=True)
                    if ch % 2 == 0:
                        nc.scalar.activation(out=yraw[:, ch, :], in_=yp[:],
                                             func=AF.Copy)
                    else:
                        nc.vector.tensor_copy(out=yraw[:, ch, :], in_=yp[:])
                s2 = bn_sums(yraw, D2, "b2")
                red2 = allreduce(s2, [D2, 2], "r2")
                a2, c2 = bn_ab(red2, gb2_sb, D2, "x2")
                apply_relu(yraw, h2[:], a2, c2)

                # ---- L3 (two halves) --------------------------------------
                h3a = sb.tile([128, P], BF, tag="slot_h13a")
                h3b = sb.tile([128, P], BF, tag="slot_h3b")
                for half, (w_sb, gb_sb, h_t, tg) in enumerate(
                    ((w3a_sb, gb3a_sb, h3a, "3a"), (w3b_sb, gb3b_sb, h3b, "3b"))
                ):
                    for ch in range(NCH):
                        yp = ps.tile([128, 512], F32, tag="yp")
                        nc.tensor.matmul(out=yp[:], lhsT=w_sb[:],
                                         rhs=h2[:, ch * 512:(ch + 1) * 512],
                                         start=True, stop=True)
                        if ch % 2 == 0:
                            nc.scalar.activation(out=yraw[:, ch, :], in_=yp[:],
                                                 func=AF.Copy)
                        else:
                            nc.vector.tensor_copy(out=yraw[:, ch, :], in_=yp[:])
                    s3 = bn_sums(yraw, 128, "b" + tg)
                    red3 = allreduce(s3, [128, 2], "r" + tg)
                    a3, c3 = bn_ab(red3, gb_sb, 128, "x" + tg)
                    apply_relu(yraw, h_t[:], a3, c3)

                # ---- L4 ---------------------------------------------------
                for grp in range(P // 1024):
                    op = psy.tile([128, 8, DO], F32, tag="op")
                    for j in range(8):
                        c = grp * 8 + j
                        nc.tensor.matmul(out=op[:, j, :],
                                         lhsT=h3a[:, c * 128:(c + 1) * 128],
                                         rhs=woa_sb[:], start=True, stop=False)
                        nc.tensor.matmul(out=op[:, j, :],
                                         lhsT=h3b[:, c * 128:(c + 1) * 128],
                                         rhs=wob_sb[:], start=False, stop=True)
                    nc.vector.tensor_tensor(out=out_sb[:, grp * 8:(grp + 1) * 8, :],
                                            in0=op[:], in1=bout_sb[:], op=OP.add)
                nc.sync.dma_start(out=out[:], in_=out_sb[:])

    nc.compile()
    return nc


def _mk_w1pt(w1_):
    wp = np.ascontiguousarray(w1_[:, 0:3].T).astype(BF16)   # [3, 128]
    z = np.zeros((6, D1), BF16)
    z[0:3] = wp
    z[3:6] = wp
    return z


def _host_prep(x, pt_loc):
    """Per-call host work: pad grid, sort points, build idx tables."""
    xpad = np.zeros((B, CIN, PGRID, PGRID, PGRID), dtype=BF16)
    xpad[:, :, 1:65, 1:65, 1:65] = x.astype(BF16)

    idx = np.clip(np.floor(pt_loc).astype(np.int64), 0, GRID - 1)
    lin = (idx[..., 0] * GRID + idx[..., 1]) * GRID + idx[..., 2]  # [B, P]
    order = np.argsort(lin, axis=1, kind="stable")
    lin_s = np.take_along_axis(lin, order, axis=1)

    gidx_l, sidx_l = [], []
    for b in range(B):
        u, first, inverse = np.unique(lin_s[b], return_index=True,
                                      return_inverse=True)
        slot = first[inverse].astype(np.int16)              # [P] < 16384
        gidx_l.append(np.tile(slot.reshape(P // 16, 16).T, (8, 1)))
        # every voxel gets a destination: real slot for first-point voxels,
        # spread trash rows >= P otherwise (mid-stream negative idx corrupt
        # the SWDGE ring accounting on HW, so scatter everything)
        scat = (P + (np.arange(NVOX) & (P - 1))).astype(np.int16)
        scat[u] = first.astype(np.int16)
        sidx_l.append(scat.reshape(NCHK, CHUNK // 16, 16).transpose(2, 0, 1))
    return xpad, order, gidx_l, sidx_l


def kernel(x, pt_loc, w3d, b3d, w1, b1, g1, beta1, w2, b2, g2, beta2,
           w3, b3, g3, beta3, w_out, b_out, **_unused):
    x = np.asarray(x, np.float32)
    pt_loc = np.asarray(pt_loc, np.float32)

    xpad, order, gidx_l, sidx_l = _host_prep(x, pt_loc)

    if _prog_cache[0] is None:
        _prog_cache[0] = _build_program()
    nc = _prog_cache[0]

    w1_ = np.asarray(w1, np.float32)
    w3d_ = np.asarray(w3d, np.float32)
    feed = {
        "ident": np.eye(128, dtype=BF16),
        # device im2col row k = ((dz*3+dy)*3+dx)*4 + cin
        "wext": np.ascontiguousarray(
            w3d_.transpose(2, 3, 4, 1, 0).reshape(K_IM, COUT)).astype(BF16),
        "b3d": np.asarray(b3d, np.float32).reshape(COUT, 1),
        "w1pt": _mk_w1pt(w1_),
        "w1cv": np.ascontiguousarray(w1_[:, 3:35].T).astype(BF16),
        "w1gl": np.ascontiguousarray(w1_[:, 35:67].T).astype(BF16),
        "w2": np.ascontiguousarray(np.asarray(w2, np.float32).T).astype(BF16),
        "w3a": np.ascontiguousarray(np.asarray(w3, np.float32)[0:128, :].T).astype(BF16),
        "w3b": np.ascontiguousarray(np.asarray(w3, np.float32)[128:256, :].T).astype(BF16),
        "wo_a": np.ascontiguousarray(np.asarray(w_out, np.float32)[:, 0:128].T).astype(BF16),
        "wo_b": np.ascontiguousarray(np.asarray(w_out, np.float32)[:, 128:256].T).astype(BF16),
        "gb1": np.stack([np.asarray(g1, np.float32), np.asarray(beta1, np.float32)], 1),
        "gb2": np.stack([np.asarray(g2, np.float32), np.asarray(beta2, np.float32)], 1),
        "gb3a": np.stack([np.asarray(g3, np.float32)[0:128],
                          np.asarray(beta3, np.float32)[0:128]], 1),
        "gb3b": np.stack([np.asarray(g3, np.float32)[128:256],
                          np.asarray(beta3, np.float32)[128:256]], 1),
        "bout": np.broadcast_to(np.asarray(b_out, np.float32), (128, 8, DO)).copy(),
    }

    in_maps = []
    for b in range(B):
        m = dict(feed)
        m["xpad"] = xpad[b]
        m["sidx"] = np.ascontiguousarray(sidx_l[b])
        m["gidx"] = np.ascontiguousarray(gidx_l[b])
        srt = np.take_along_axis(pt_loc[b], order[b][:, None], axis=0)
        sT = srt.T                                   # [3, P] f32
        hi = np.round(sT * 4.0) / 4.0                # exact in bf16 (< 64, res 0.25)
        lo = (sT - hi).astype(BF16)
        pt3 = np.zeros((6, P), BF16)
        pt3[0:3] = hi.astype(BF16)
        pt3[3:6] = lo
        m["ptT"] = pt3
        in_maps.append(m)

    try:
        results = _run_cached(nc, in_maps)
    except Exception:
        _runner_cache[0] = None
        results = run_bass_kernel_spmd(nc, in_maps,
                                       core_ids=list(range(8))).results

    out = np.zeros((B, DO, P), np.float32)
    for b in range(B):
        ob = results[b]["out"]                           # [128, P//128, 10]
        flat = ob.transpose(1, 0, 2).reshape(P, DO)      # col q = c*128+p
        inv = np.empty(P, np.int64)
        inv[order[b]] = np.arange(P)
        out[b] = flat[inv].T
    return out


# revision 15
# speedup vs baseline: 1.2085x; 1.2085x over previous
"""Trainium2 Bass kernel for voxel-CNN + point-MLP (nn_CNN_Baseline_62646392980178).

Sharding: data-parallel over batch B=8 across 8 NeuronCores (one sample per
core); params replicated.

Front-end (this rewrite): instead of packing 256B neighborhood rows on the
host (512MB of numpy + ~1GB host->device traffic), each core receives only
the raw zero-padded voxel grid (bf16 [4,66,66,66], 2.3MB).  On device, the
full-grid conv runs as im2col built by 27 structured DMAs per voxel chunk
(one per 3x3x3 tap, 4 input channels each) followed by one [108->32] matmul.
Conv columns are PE-transposed to per-voxel 256B rows and dma_scatter_add'ed
into a small DRAM table at "first point of this voxel" slots (idx -1 = no
point -> skipped).  A single transpose-mode dma_gather (idx = first-point
slot of each sorted point, which also resolves duplicate-voxel points) then
yields conv features as [channel, point] columns feeding the MLP directly.

Back-end (unchanged from baseline): MLP (128/128/256/10) on TensorE over the
sorted points; training-mode BatchNorm over (B, P) via per-channel sum
allreduce across the 8 cores; the global-max feature's layer-1 contribution
is folded into the BN statistics and layer-1 bias (cross terms in the
allreduce payload), removing the max-pool barrier from the matmul pipeline.
"""

import os
import sys

sys.path.insert(0, "/opt/trn_rl_repo")

import numpy as np
import ml_dtypes

import concourse.bass as bass
import concourse.bacc as bacc
import concourse.mybir as mybir
import concourse.tile as tile
from concourse.bass_utils import run_bass_kernel_spmd

BF16 = ml_dtypes.bfloat16
F32 = mybir.dt.float32
BF = mybir.dt.bfloat16
I16 = mybir.dt.int16
AF = mybir.ActivationFunctionType
OP = mybir.AluOpType

GRID = 64
PGRID = GRID + 2
NVOX = GRID ** 3
EPS = 1e-5
B = 8
P = 16384
CIN = 4
COUT = 32
K_IM = 108
ROW = 128             # bf16 elems per table row (256B)
NSLOT = 32768         # conv table rows; [P, 32768) = trash (never gathered)
CHUNK = 2048          # voxels per conv chunk (half a z-plane)
NCHK = NVOX // CHUNK  # 128
YROWS = CHUNK // GRID  # 32 y-rows per chunk
NCH = P // 512        # 32 MLP chunks
D1, D2, D3, DO = 128, 128, 256, 10
N_TOT = float(B * P)

_prog_cache = [None]
_runner_cache = [None]
_REPLICATED = frozenset([
    "ident", "wext", "b3d", "w1pt", "w1cv", "w1gl", "w2", "w3a", "w3b",
    "wo_a", "wo_b", "gb1", "gb2", "gb3a", "gb3b", "bout"])


def _run_cached(nc, in_maps):
    """Dispatch the prebuilt Bass module via a PERSISTENT jitted callable.

    run_bass_kernel_spmd -> run_bass_via_pjrt builds a fresh jax.jit wrapper
    on every call (full retrace + XLA pipeline each time, ~1s).  This caches
    the sharded executable across kernel() calls; logic mirrors
    bass2jax.run_bass_via_pjrt's multi-core branch.
    """
    import jax
    from jax.experimental.shard_map import shard_map
    from jax.sharding import Mesh, PartitionSpec
    from concourse import bass2jax as B2J

    if _runner_cache[0] is None:
        B2J.install_neuronx_cc_hook()
        assert nc.dbg_addr is None, "cached runner assumes debug=False"
        partition_name = (nc.partition_id_tensor.name
                          if nc.partition_id_tensor else None)
        in_names, out_names, out_avals = [], [], []
        for alloc in nc.m.functions[0].allocations:
            if not isinstance(alloc, mybir.MemoryLocationSet):
                continue
            name = alloc.memorylocations[0].name
            if alloc.kind == "ExternalInput":
                if name != partition_name:
                    in_names.append(name)
            elif alloc.kind == "ExternalOutput":
                out_names.append(name)
                out_avals.append(jax.core.ShapedArray(
                    tuple(alloc.tensor_shape), mybir.dt.np(alloc.dtype)))
        n_params, n_outs = len(in_names), len(out_avals)
        all_names = list(in_names) + list(out_names)
        if partition_name is not None:
            all_names.append(partition_name)
        donate = tuple(range(n_params, n_params + n_outs))

        def _body(*args):
            operands = list(args)
            if partition_name is not None:
                operands.append(B2J.partition_id_tensor())
            return tuple(B2J._bass_exec_p.bind(
                *operands, out_avals=tuple(out_avals),
                in_names=tuple(all_names), out_names=tuple(out_names),
                lowering_input_output_aliases=(),
                sim_require_finite=True, sim_require_nnan=True, nc=nc))

        devices = jax.devices()[:B]
        mesh = Mesh(np.asarray(devices), ("core",))
        in_specs = tuple(
            PartitionSpec() if name in _REPLICATED else PartitionSpec("core")
            for name in in_names) + (PartitionSpec("core"),) * n_outs
        sharded = jax.jit(
            shard_map(_body, mesh=mesh, in_specs=in_specs,
                      out_specs=(PartitionSpec("core"),) * n_outs,
                      check_rep=False),
            donate_argnums=donate, keep_unused=True)
        _runner_cache[0] = (sharded, in_names, out_names, out_avals)

    sharded, in_names, out_names, out_avals = _runner_cache[0]
    concat_in = [
        in_maps[0][name] if name in _REPLICATED
        else np.concatenate([np.asarray(m[name]) for m in in_maps], axis=0)
        for name in in_names]
    concat_zeros = [np.zeros((B * a.shape[0], *a.shape[1:]), a.dtype)
                    for a in out_avals]
    out_arrs = sharded(*concat_in, *concat_zeros)
    return [{name: np.asarray(out_arrs[i]).reshape(B, *out_avals[i].shape)[c]
             for i, name in enumerate(out_names)}
            for c in range(B)]


def _build_program():
    nc = bacc.Bacc("TRN2", target_bir_lowering=False, debug=False, num_devices=8)

    def din(name, shape, dt):
        return nc.dram_tensor(name, shape, dt, kind="ExternalInput").ap()

    xpad = din("xpad", [CIN, PGRID, PGRID, PGRID], BF)
    sidx = din("sidx", [16, NCHK, CHUNK // 16], I16)
    gidx = din("gidx", [16, P // 16], I16)
    ptT = din("ptT", [6, P], BF)
    ident = din("ident", [128, 128], BF)
    wext = din("wext", [K_IM, COUT], BF)
    b3dp = din("b3d", [COUT, 1], F32)
    w1pt = din("w1pt", [6, D1], BF)
    w1cv = din("w1cv", [COUT, D1], BF)
    w1gl = din("w1gl", [COUT, D1], BF)
    w2p = din("w2", [D1, D2], BF)
    w3ap = din("w3a", [D2, 128], BF)
    w3bp = din("w3b", [D2, 128], BF)
    woap = din("wo_a", [128, DO], BF)
    wobp = din("wo_b", [128, DO], BF)
    gb1p = din("gb1", [D1, 2], F32)
    gb2p = din("gb2", [D2, 2], F32)
    gb3ap = din("gb3a", [128, 2], F32)
    gb3bp = din("gb3b", [128, 2], F32)
    boutp = din("bout", [128, 8, DO], F32)
    out = nc.dram_tensor("out", [128, P // 128, DO], BF, kind="ExternalOutput").ap()

    _stage = os.environ.get("K_STAGE", "full")
    _skip_cc = os.environ.get("K_SKIP_CC", "0") == "1"

    with tile.TileContext(nc) as tc:
        with tc.tile_pool(name="sb", bufs=1) as sb, \
             tc.tile_pool(name="ps", bufs=2, space="PSUM") as ps, \
             tc.tile_pool(name="psy", bufs=1, space="PSUM") as psy, \
             tc.tile_pool(name="imp", bufs=2) as imp, \
             tc.tile_pool(name="ssp", bufs=2) as ssp, \
             tc.tile_pool(name="sxp", bufs=2) as sxp, \
             tc.tile_pool(name="dramp", bufs=1, space="DRAM") as dramp:

            table = dramp.tile([NSLOT, ROW], BF, tag="table")

            gidx_sb = sb.tile([128, P // 16], I16, tag="gidx")
            nc.sync.dma_start(out=gidx_sb[0:16, :], in_=gidx[:])
            for r in range(1, 8):
                nc.sync.dma_start(out=gidx_sb[r * 16:(r + 1) * 16, :],
                                  in_=gidx_sb[0:16, :])

            id_sb = sb.tile([128, 128], BF, tag="ident")
            nc.sync.dma_start(out=id_sb[:], in_=ident[:])

            def loadw(ap_, shape, dt, tag):
                t = sb.tile(shape, dt, tag=tag)
                nc.sync.dma_start(out=t[:], in_=ap_[:])
                return t

            wext_sb = loadw(wext, [K_IM, COUT], BF, "wext")
            b3d_sb = loadw(b3dp, [COUT, 1], F32, "b3d")
            w1pt_sb = loadw(w1pt, [6, D1], BF, "w1pt")
            w1cv_sb = loadw(w1cv, [COUT, D1], BF, "w1cv")
            w1gl_sb = loadw(w1gl, [COUT, D1], BF, "w1gl")
            w2_sb = loadw(w2p, [D1, D2], BF, "w2")
            w3a_sb = loadw(w3ap, [D2, 128], BF, "w3a")
            w3b_sb = loadw(w3bp, [D2, 128], BF, "w3b")
            woa_sb = loadw(woap, [128, DO], BF, "woa")
            wob_sb = loadw(wobp, [128, DO], BF, "wob")
            gb1_sb = loadw(gb1p, [D1, 2], F32, "gb1")
            gb2_sb = loadw(gb2p, [D2, 2], F32, "gb2")
            gb3a_sb = loadw(gb3ap, [128, 2], F32, "gb3a")
            gb3b_sb = loadw(gb3bp, [128, 2], F32, "gb3b")
            bout_sb = loadw(boutp, [128, 8, DO], F32, "bout")

            # ---- zero the scatter table (scatter_add accumulates) ----------
            zt = sb.tile([128, 1032], BF, tag="zt")
            nc.vector.memset(zt[:], 0.0)
            for i in range(16):
                nc.sync.dma_start(out=table[i * 1032:(i + 1) * 1032, :],
                                  in_=zt[:])

            # ---- full-grid conv -> per-voxel rows -> scatter to slots ------
            _nchk = int(os.environ.get("K_NCHK", str(NCHK)))
            for ck in range(_nchk if _stage != "mlponly" else 0):
                z = ck // 2
                y0 = (ck % 2) * YROWS
                im = imp.tile([K_IM, CHUNK], BF, tag="im")
                for dz in range(3):
                    for dy in range(3):
                        for dx in range(3):
                            k0 = ((dz * 3 + dy) * 3 + dx) * CIN
                            nc.sync.dma_start(
                                out=im[k0:k0 + CIN, :],
                                in_=xpad[0:CIN, z + dz,
                                         y0 + dy:y0 + dy + YROWS,
                                         dx:dx + GRID])
                ssrc = ssp.tile([128, CHUNK // 128, ROW], BF, tag="ssrc")
                if ck < 2:
                    nc.vector.memset(ssrc[:], 0.0)
                for i in range(CHUNK // 512):
                    cp = ps.tile([COUT, 512], F32, tag="cp")
                    nc.tensor.matmul(out=cp[:], lhsT=wext_sb[:],
                                     rhs=im[:, i * 512:(i + 1) * 512],
                                     start=True, stop=True)
                    cv = ssp.tile([COUT, 512], BF, tag="cv")
                    if i % 2 == 0:
                        nc.scalar.activation(out=cv[:], in_=cp[:], func=AF.Copy)
                    else:
                        nc.vector.tensor_copy(out=cv[:], in_=cp[:])
                    tp = ps.tile([128, 4, COUT], BF, tag="tp")
                    for t in range(4):
                        nc.tensor.transpose(
                            out=tp[:, t, :],
                            in_=cv[:, t * 128:(t + 1) * 128],
                            identity=id_sb[0:COUT, 0:COUT])
                    if i % 2 == 0:
                        nc.vector.tensor_copy(
                            out=ssrc[:, i * 4:(i + 1) * 4, 0:COUT], in_=tp[:])
                    else:
                        nc.scalar.activation(
                            out=ssrc[:, i * 4:(i + 1) * 4, 0:COUT], in_=tp[:],
                            func=AF.Copy)
                sx = sxp.tile([128, CHUNK // 16], I16, tag="sx")
                for j in range(8):
                    nc.sync.dma_start(out=sx[j * 16:(j + 1) * 16, :],
                                      in_=sidx[:, ck, :])
                for h in range(CHUNK // 512):
                    nc.gpsimd.dma_scatter_add(
                        out_ap=table[:], in_ap=ssrc[:, h * 4:(h + 1) * 4, :],
                        idxs_ap=sx[:, h * 32:(h + 1) * 32],
                        num_idxs=512, num_idxs_reg=512, elem_size=ROW,
                        queue_num=0)

            # ---- gather conv at sorted points: cpt[e, p] = table[gidx[p], e]
            cpt = sb.tile([128, 1, P], BF, tag="slot_cpt")
            if os.environ.get("K_NOGATHER", "0") != "1":
                for g in range(int(os.environ.get("K_NGCALLS", str(P // 512)))):
                    nc.gpsimd.dma_gather(
                        out_ap=cpt[:, :, g * 512:(g + 1) * 512],
                        in_ap=table[:],
                        idxs_ap=gidx_sb[:, g * 32:(g + 1) * 32],
                        num_idxs=512, num_idxs_reg=512, elem_size=ROW,
                        transpose=True, queue_num=0)
            else:
                nc.vector.memset(cpt[:], 0.25)
            conv_sb = cpt[0:COUT, 0, :]

            yraw = sb.tile([128, NCH, 512], F32, tag="slot_yraw")
            h1 = sb.tile([D1, P], BF, tag="slot_h13a")
            out_sb = sb.tile([128, P // 128, DO], BF, tag="outsb")

            if _stage == "conv":
                nc.vector.memset(out_sb[:], 0.0)
                nc.vector.tensor_copy(out=out_sb[:, 0:8, :],
                                      in_=cpt[:, 0, 0:80])
                nc.sync.dma_start(out=out[:], in_=out_sb[:])
            if _stage == "full":
                # ---- glob max -> v_g ---------------------------------------
                glob = sb.tile([COUT, 1], BF, tag="glob")
                nc.vector.tensor_reduce(out=glob[:], in_=conv_sb,
                                        axis=mybir.AxisListType.X, op=OP.max)
                nc.vector.tensor_scalar(out=glob[:], in0=glob[:],
                                        scalar1=b3d_sb[:, 0:1], scalar2=None,
                                        op0=OP.add)
                vgp = psy.tile([D1, 1], F32, tag="vg")
                nc.tensor.matmul(out=vgp[:], lhsT=w1gl_sb[:],
                                 rhs=glob[:], start=True, stop=True)
                vg = sb.tile([D1, 1], F32, tag="vgs")
                nc.vector.tensor_copy(out=vg[:], in_=vgp[:])

                # ---- L1 ----------------------------------------------------
                ptb = None
                for ch in range(NCH):
                    if ch % 4 == 0:
                        ptb = imp.tile([6, 2048], BF, tag="ptb")
                        nc.sync.dma_start(out=ptb[:],
                                          in_=ptT[:, ch * 512:(ch + 4) * 512])
                    yp = ps.tile([D1, 512], F32, tag="yp")
                    nc.tensor.matmul(out=yp[:],
                                     lhsT=w1pt_sb[:],
                                     rhs=ptb[:, (ch % 4) * 512:(ch % 4 + 1) * 512],
                                     start=True, stop=False)
                    nc.tensor.matmul(out=yp[:], lhsT=w1cv_sb[:],
                                     rhs=conv_sb[:, ch * 512:(ch + 1) * 512],
                                     start=False, stop=True)
                    if ch % 2 == 0:
                        nc.vector.tensor_copy(out=yraw[:, ch, :], in_=yp[:])
                    else:
                        nc.scalar.activation(out=yraw[:, ch, :], in_=yp[:],
                                             func=AF.Copy)

                # ---- helpers ----------------------------------------------
                def bn_sums(y_t, C, tag):
                    st = sb.tile([C, NCH * 6], F32, tag=tag + "st")
                    for ch in range(NCH):
                        nc.vector.bn_stats(out=st[:, ch * 6:(ch + 1) * 6],
                                           in_=y_t[:, ch, :])
                    ag = sb.tile([C, 2], F32, tag=tag + "ag")
                    nc.vector.bn_aggr(out=ag[:], in_=st[:])
                    s = sb.tile([C, 2], F32, tag=tag + "s")
                    nc.vector.tensor_tensor(out=s[:, 1:2], in0=ag[:, 0:1],
                                            in1=ag[:, 0:1], op=OP.mult)
                    nc.vector.tensor_tensor(out=s[:, 1:2], in0=s[:, 1:2],
                                            in1=ag[:, 1:2], op=OP.add)
                    nc.vector.tensor_scalar_mul(out=s[:, 1:2], in0=s[:, 1:2],
                                                scalar1=float(P))
                    nc.vector.tensor_scalar_mul(out=s[:, 0:1], in0=ag[:, 0:1],
                                                scalar1=float(P))
                    return s

                def allreduce(t_sb, shape, tag):
                    if _skip_cc:
                        r = sb.tile(shape, F32, tag=tag + "r")
                        nc.vector.tensor_scalar_mul(out=r[:], in0=t_sb[:],
                                                    scalar1=8.0)
                        return r
                    di = dramp.tile(shape, F32, tag=tag + "i")
                    do = dramp.tile(shape, F32, tag=tag + "o")
                    nc.gpsimd.dma_start(out=di[:], in_=t_sb[:])
                    nc.gpsimd.collective_compute(
                        "AllReduce", OP.add, replica_groups=[list(range(8))],
                        ins=[di.opt()], outs=[do.opt()])
                    r = sb.tile(shape, F32, tag=tag + "r")
                    nc.sync.dma_start(out=r[:], in_=do[:])
                    return r

                def bn_ab(red, gb, C, tag, vg_local=None):
                    a = sb.tile([C, 1], F32, tag=tag + "a")
                    cb = sb.tile([C, 1], F32, tag=tag + "c")
                    m = sb.tile([C, 1], F32, tag=tag + "m")
                    if vg_local is None:
                        nc.vector.tensor_scalar_mul(out=m[:], in0=red[:, 0:1],
                                                    scalar1=1.0 / N_TOT)
                        nc.vector.tensor_scalar_mul(out=a[:], in0=red[:, 1:2],
                                                    scalar1=1.0 / N_TOT)
                    else:
                        nc.vector.tensor_scalar_mul(out=m[:], in0=red[:, 4:5],
                                                    scalar1=float(P))
                        nc.vector.tensor_tensor(out=m[:], in0=m[:],
                                                in1=red[:, 0:1], op=OP.add)
                        nc.vector.tensor_scalar_mul(out=m[:], in0=m[:],
                                                    scalar1=1.0 / N_TOT)
                        t4 = sb.tile([C, 1], F32, tag=tag + "t4")
                        nc.vector.tensor_scalar_mul(out=a[:], in0=red[:, 2:3],
                                                    scalar1=2.0)
                        nc.vector.tensor_tensor(out=a[:], in0=a[:],
                                                in1=red[:, 1:2], op=OP.add)
                        nc.vector.tensor_scalar_mul(out=t4[:], in0=red[:, 3:4],
                                                    scalar1=float(P))
                        nc.vector.tensor_tensor(out=a[:], in0=a[:], in1=t4[:],
                                                op=OP.add)
                        nc.vector.tensor_scalar_mul(out=a[:], in0=a[:],
                                                    scalar1=1.0 / N_TOT)
                    msq = sb.tile([C, 1], F32, tag=tag + "q")
                    nc.vector.tensor_tensor(out=msq[:], in0=m[:], in1=m[:],
                                            op=OP.mult)
                    nc.vector.tensor_tensor(out=a[:], in0=a[:], in1=msq[:],
                                            op=OP.subtract)
                    nc.vector.tensor_scalar_add(out=a[:], in0=a[:], scalar1=EPS)
                    nc.scalar.activation(out=a[:], in_=a[:], func=AF.Sqrt)
                    nc.vector.reciprocal(out=a[:], in_=a[:])
                    nc.vector.tensor_tensor(out=a[:], in0=a[:], in1=gb[:, 0:1],
                                            op=OP.mult)
                    if vg_local is not None:
                        nc.vector.tensor_tensor(out=cb[:], in0=vg_local[:],
                                                in1=m[:], op=OP.subtract)
                        nc.vector.tensor_tensor(out=cb[:], in0=cb[:], in1=a[:],
                                                op=OP.mult)
                    else:
                        nc.vector.tensor_tensor(out=cb[:], in0=m[:], in1=a[:],
                                                op=OP.mult)
                        nc.vector.tensor_scalar_mul(out=cb[:], in0=cb[:],
                                                    scalar1=-1.0)
                    nc.vector.tensor_tensor(out=cb[:], in0=cb[:], in1=gb[:, 1:2],
                                            op=OP.add)
                    return a, cb

                def apply_relu(y_t, h_ap, a, cb):
                    hf = NCH // 2
                    nc.scalar.activation(out=h_ap[:, 0:P // 2],
                                         in_=y_t[:, 0:hf, :],
                                         func=AF.Relu, bias=cb[:], scale=a[:])
                    nc.vector.tensor_scalar(out=h_ap[:, P // 2:P],
                                            in0=y_t[:, hf:NCH, :],
                                            scalar1=a[:], scalar2=cb[:],
                                            op0=OP.mult, op1=OP.add)
                    nc.vector.tensor_scalar_max(out=h_ap[:, P // 2:P],
                                                in0=h_ap[:, P // 2:P],
                                                scalar1=0.0)

                # ---- BN1 ---------------------------------------------------
                s1 = bn_sums(yraw, D1, "b1")
                pk1 = sb.tile([D1, 5], F32, tag="pk1")
                nc.vector.tensor_copy(out=pk1[:, 0:2], in_=s1[:])
                nc.vector.tensor_tensor(out=pk1[:, 2:3], in0=vg[:],
                                        in1=s1[:, 0:1], op=OP.mult)
                nc.vector.tensor_tensor(out=pk1[:, 3:4], in0=vg[:], in1=vg[:],
                                        op=OP.mult)
                nc.vector.tensor_copy(out=pk1[:, 4:5], in_=vg[:])
                red1 = allreduce(pk1, [D1, 5], "r1")
                a1, c1 = bn_ab(red1, gb1_sb, D1, "x1", vg_local=vg)
                apply_relu(yraw, h1[:], a1, c1)

                # ---- L2 ----------------------------------------------------
                h2 = sb.tile([D2, P], BF, tag="slot_cpt")
                for ch in range(NCH):
                    yp = ps.tile([D2, 512], F32, tag="yp")
                    nc.tensor.matmul(out=yp[:], lhsT=w2_sb[:],
                                     rhs=h1[:, ch * 512:(ch + 1) * 512],
                                     start=True, stop=True)
                    if ch % 2 == 0:
                        nc.scalar.activation(out=yraw[:, ch, :], in_=yp[:],
                                             func=AF.Copy)
                    else:
                        nc.vector.tensor_copy(out=yraw[:, ch, :], in_=yp[:])
                s2 = bn_sums(yraw, D2, "b2")
                red2 = allreduce(s2, [D2, 2], "r2")
                a2, c2 = bn_ab(red2, gb2_sb, D2, "x2")
                apply_relu(yraw, h2[:], a2, c2)

                # ---- L3 (two halves) --------------------------------------
                h3a = sb.tile([128, P], BF, tag="slot_h13a")
                h3b = sb.tile([128, P], BF, tag="slot_h3b")
                for half, (w_sb, gb_sb, h_t, tg) in enumerate(
                    ((w3a_sb, gb3a_sb, h3a, "3a"), (w3b_sb, gb3b_sb, h3b, "3b"))
                ):
                    for ch in range(NCH):
                        yp = ps.tile([128, 512], F32, tag="yp")
                        nc.tensor.matmul(out=yp[:], lhsT=w_sb[:],
                                         rhs=h2[:, ch * 512:(ch + 1) * 512],
                                         start=True, stop=True)
                        if ch % 2 == 0:
                            nc.scalar.activation(out=yraw[:, ch, :], in_=yp[:],
                                                 func=AF.Copy)
                        else:
                            nc.vector.tensor_copy(out=yraw[:, ch, :], in_=yp[:])
                    s3 = bn_sums(yraw, 128, "b" + tg)
                    red3 = allreduce(s3, [128, 2], "r" + tg)
                    a3, c3 = bn_ab(red3, gb_sb, 128, "x" + tg)
                    apply_relu(yraw, h_t[:], a3, c3)

                # ---- L4 ---------------------------------------------------
                for grp in range(P // 1024):
                    op = psy.tile([128, 8, DO], F32, tag="op")
                    for j in range(8):
                        c = grp * 8 + j
                        nc.tensor.matmul(out=op[:, j, :],
                                         lhsT=h3a[:, c * 128:(c + 1) * 128],
                                         rhs=woa_sb[:], start=True, stop=False)
                        nc.tensor.matmul(out=op[:, j, :],
                                         lhsT=h3b[:, c * 128:(c + 1) * 128],
                                         rhs=wob_sb[:], start=False, stop=True)
                    nc.vector.tensor_tensor(out=out_sb[:, grp * 8:(grp + 1) * 8, :],
                                            in0=op[:], in1=bout_sb[:], op=OP.add)
                nc.sync.dma_start(out=out[:], in_=out_sb[:])

    nc.compile()
    return nc


def _mk_w1pt(w1_):
    wp = np.ascontiguousarray(w1_[:, 0:3].T).astype(BF16)   # [3, 128]
    z = np.zeros((6, D1), BF16)
    z[0:3] = wp
    z[3:6] = wp
    return z


def _host_prep(x, pt_loc):
    """Per-call host work: pad grid, sort points, build idx tables."""
    xpad = np.zeros((B, CIN, PGRID, PGRID, PGRID), dtype=BF16)
    xpad[:, :, 1:65, 1:65, 1:65] = x.astype(BF16)

    idx = np.clip(np.floor(pt_loc).astype(np.int64), 0, GRID - 1)
    lin = (idx[..., 0] * GRID + idx[..., 1]) * GRID + idx[..., 2]  # [B, P]
    order = np.argsort(lin, axis=1, kind="stable")
    lin_s = np.take_along_axis(lin, order, axis=1)

    gidx_l, sidx_l = [], []
    for b in range(B):
        u, first, inverse = np.unique(lin_s[b], return_index=True,
                                      return_inverse=True)
        slot = first[inverse].astype(np.int16)              # [P] < 16384
        gidx_l.append(slot.reshape(P // 16, 16).T)
        # every voxel gets a destination: real slot for first-point voxels,
        # spread trash rows >= P otherwise (mid-stream negative idx corrupt
        # the SWDGE ring accounting on HW, so scatter everything)
        scat = (P + (np.arange(NVOX) & (P - 1))).astype(np.int16)
        scat[u] = first.astype(np.int16)
        sidx_l.append(scat.reshape(NCHK, CHUNK // 16, 16).transpose(2, 0, 1))
    return xpad, order, gidx_l, sidx_l


def kernel(x, pt_loc, w3d, b3d, w1, b1, g1, beta1, w2, b2, g2, beta2,
           w3, b3, g3, beta3, w_out, b_out, **_unused):
    x = np.asarray(x, np.float32)
    pt_loc = np.asarray(pt_loc, np.float32)

    xpad, order, gidx_l, sidx_l = _host_prep(x, pt_loc)

    if _prog_cache[0] is None:
        _prog_cache[0] = _build_program()
    nc = _prog_cache[0]

    w1_ = np.asarray(w1, np.float32)
    w3d_ = np.asarray(w3d, np.float32)
    feed = {
        "ident": np.eye(128, dtype=BF16),
        # device im2col row k = ((dz*3+dy)*3+dx)*4 + cin
        "wext": np.ascontiguousarray(
            w3d_.transpose(2, 3, 4, 1, 0).reshape(K_IM, COUT)).astype(BF16),
        "b3d": np.asarray(b3d, np.float32).reshape(COUT, 1),
        "w1pt": _mk_w1pt(w1_),
        "w1cv": np.ascontiguousarray(w1_[:, 3:35].T).astype(BF16),
        "w1gl": np.ascontiguousarray(w1_[:, 35:67].T).astype(BF16),
        "w2": np.ascontiguousarray(np.asarray(w2, np.float32).T).astype(BF16),
        "w3a": np.ascontiguousarray(np.asarray(w3, np.float32)[0:128, :].T).astype(BF16),
        "w3b": np.ascontiguousarray(np.asarray(w3, np.float32)[128:256, :].T).astype(BF16),
        "wo_a": np.ascontiguousarray(np.asarray(w_out, np.float32)[:, 0:128].T).astype(BF16),
        "wo_b": np.ascontiguousarray(np.asarray(w_out, np.float32)[:, 128:256].T).astype(BF16),
        "gb1": np.stack([np.asarray(g1, np.float32), np.asarray(beta1, np.float32)], 1),
        "gb2": np.stack([np.asarray(g2, np.float32), np.asarray(beta2, np.float32)], 1),
        "gb3a": np.stack([np.asarray(g3, np.float32)[0:128],
                          np.asarray(beta3, np.float32)[0:128]], 1),
        "gb3b": np.stack([np.asarray(g3, np.float32)[128:256],
                          np.asarray(beta3, np.float32)[128:256]], 1),
        "bout": np.broadcast_to(np.asarray(b_out, np.float32), (128, 8, DO)).copy(),
    }

    in_maps = []
    for b in range(B):
        m = dict(feed)
        m["xpad"] = xpad[b]
        m["sidx"] = np.ascontiguousarray(sidx_l[b])
        m["gidx"] = np.ascontiguousarray(gidx_l[b])
        srt = np.take_along_axis(pt_loc[b], order[b][:, None], axis=0)
        sT = srt.T                                   # [3, P] f32
        hi = np.round(sT * 4.0) / 4.0                # exact in bf16 (< 64, res 0.25)
        lo = (sT - hi).astype(BF16)
        pt3 = np.zeros((6, P), BF16)
        pt3[0:3] = hi.astype(BF16)
        pt3[3:6] = lo
        m["ptT"] = pt3
        in_maps.append(m)

    try:
        results = _run_cached(nc, in_maps)
    except Exception:
        _runner_cache[0] = None
        results = run_bass_kernel_spmd(nc, in_maps,
                                       core_ids=list(range(8))).results

    out = np.zeros((B, DO, P), np.float32)
    for b in range(B):
        ob = np.asarray(results[b]["out"], np.float32)   # [128, P//128, 10] bf16
        flat = ob.transpose(1, 0, 2).reshape(P, DO)      # col q = c*128+p
        inv = np.empty(P, np.int64)
        inv[order[b]] = np.arange(P)
        out[b] = flat[inv].T
    return out
